# revision 19
# baseline (speedup 1.0000x reference)
"""ChildSumTreeLSTM (N=8192 complete 8-ary tree) on 8 TRN2 NeuronCores.

Decomposition (all tree structure is compile-time static):
- nodes 0..1023 are internal (children of p = 8p+1..8p+8), 1024..8191 leaves.
- Phase A (per core): iou_x/fx_x projections for the ~1096 node-columns this
  core owns, feature-major, fp16 matmuls on the PE (f32 PSUM accumulate).
- Leaf phase: elementwise sigmoid/tanh -> leaf (h, c), spilled to DRAM (fp16).
- 5 sequential rounds of internal levels: R4 (parents 585..1023, 439),
  R3 (73..584, 512), R2 (9..72, 64), R1 (1..8, 8), R0 (root).
  Each round is node-sharded across the 8 cores so that every child a core
  needs was computed locally, except: R4 results are AllGather'ed (core 0
  consumes them for R3), and R1 results are AllGather'ed (every core then
  computes the root; core 0's answer is returned).
"""
import sys
import functools

sys.path.insert(0, '/opt/trn_rl_repo')

import numpy as np
import concourse.bacc as bacc
import concourse.mybir as mybir
import concourse.tile as tile
from concourse.bass_utils import run_bass_kernel_spmd

DT = mybir.dt
AF = mybir.ActivationFunctionType
F16 = DT.float16

NCORES = 8
N = 8192
M = 1024
C4 = [54, 54, 55, 55, 55, 55, 55, 56]
S4 = [585, 639, 693, 748, 803, 858, 913, 968]
NB = [56, 64, 8, 1, 1]          # parents per round (uniform per core)
Q0 = [0, 56, 120, 128, 129]     # row offset into the node-major parent table
NCOLS = 1096                    # 448 (R4 children) + 512 (R3 children) + 136 parents
QW = 274                        # quarter-block width (4 quarters)


def _core_cols(i):
    cols = []
    for pl in range(56):
        for k in range(8):
            if pl < C4[i]:
                node = 8 * (S4[i] + pl) + 1 + k
                cols.append(node if node < N else -1)
            else:
                cols.append(-1)
    for b in range(512):
        node = 585 + 512 * i + b
        cols.append(-1 if (i == 0 and b < 439) else node)
    for q in range(56):
        cols.append(S4[i] + q if q < C4[i] else -1)
    cols += [73 + 64 * i + j for j in range(64)]
    cols += [9 + 8 * i + j for j in range(8)]
    cols += [1 + i, 0] + [-1] * 6
    return cols


@functools.lru_cache(maxsize=1)
def _build():
    nc = bacc.Bacc(trn_type="TRN2", target_bir_lowering=False, debug=False,
                   num_devices=NCORES)

    xT_d = nc.dram_tensor("xT", [4, 128, 8 * QW], F16, kind="ExternalInput")
    WAG_d = nc.dram_tensor("WAG", [8, 128, 4096], F16, kind="ExternalInput")
    WRG_d = nc.dram_tensor("WRG", [8, 128, 4096], F16, kind="ExternalInput")
    BT_d = nc.dram_tensor("BT", [128, 32], DT.float32, kind="ExternalInput")
    SEL_d = nc.dram_tensor("SEL", [128, 1024], F16, kind="ExternalInput")
    I_d = nc.dram_tensor("I128", [128, 128], F16, kind="ExternalInput")
    rh_d = nc.dram_tensor("root_h", [1, M], DT.float32, kind="ExternalOutput")
    rc_d = nc.dram_tensor("root_c", [1, M], DT.float32, kind="ExternalOutput")

    RG = [list(range(NCORES))]

    with tile.TileContext(nc) as tc:
        with (
            tc.tile_pool(name="dram", bufs=1, space="DRAM") as dram,
            tc.tile_pool(name="persist", bufs=1) as pp,
            tc.tile_pool(name="wpool", bufs=1) as wp,
        ):
            spill4 = dram.tile([2, 128, 8, 448], F16)
            spill3 = dram.tile([2, 128, 8, 512], F16)
            nm_dram = dram.tile([136, 4096], F16)
            ag_in = dram.tile([2, 128, 8, 56], F16)
            ag_out = dram.tile([NCORES, 2, 128, 8, 56], F16,
                               addr_space="Shared")
            agb_in = dram.tile([2, 128, 8, 1], F16)
            agb_out = dram.tile([NCORES, 2, 128, 8, 1], F16,
                                addr_space="Shared")

            I_t = pp.tile([128, 128], F16)
            SEL_t = pp.tile([128, 1024], F16)
            BT_t = pp.tile([128, 32], DT.float32)
            zpad = pp.tile([128, 8], F16)

            # weight chunks: WAg[jm][p, k, gi*128+c] (Phase A), WRg[ch] (rounds)
            WAg = [wp.tile([128, 8, 512], F16, tag=f"wa{j}", bufs=1,
                           name=f"WAg{j}") for j in range(8)]
            WRg = [wp.tile([128, 8, 512], F16, tag=f"wr{j}", bufs=1,
                           name=f"WRg{j}") for j in range(8)]

            # ---------------- Phase A + leaves ----------------
            with (
                tc.tile_pool(name="xp", bufs=1) as xp,
                tc.tile_pool(name="pap", bufs=1, space="PSUM") as pap,
                tc.tile_pool(name="drp", bufs=1) as drp,
                tc.tile_pool(name="stp", bufs=1) as stp,
            ):
                xsq = [xp.tile([128, 8, QW], F16, tag=f"xs{q}", bufs=1,
                               name=f"xsq{q}") for q in range(4)]
                # DMA issue order: x quarters, biases, Phase-A weights (in use
                # order), identity/SEL, then round weights.
                engs = [nc.sync, nc.scalar, nc.gpsimd]
                for q in range(4):
                    engs[q % 3].dma_start(
                        xsq[q][:], xT_d[q].rearrange("p (k w) -> p k w", k=8))
                nc.gpsimd.dma_start(BT_t[:], BT_d[:])
                for j in range(8):
                    engs[j % 3].dma_start(
                        WAg[j][:], WAG_d[j].rearrange("p (k c) -> p k c", k=8))
                nc.gpsimd.dma_start(I_t[:], I_d[:])
                nc.gpsimd.dma_start(SEL_t[:], SEL_d[:])
                nc.vector.memset(zpad[:], 0.0)

                def load_wr(j):
                    nc.sync.dma_start(
                        WRg[j][:], WRG_d[j].rearrange("p (k c) -> p k c", k=8))

                for b4 in range(4):
                    for jm in range(8):
                        ps = {}
                        for gi in range(3):
                            ps[gi] = pap.tile([128, QW], DT.float32, tag="pa",
                                              bufs=8, name=f"pa_{jm}_{b4}_{gi}")
                        psf = None
                        if b4 == 3:
                            psf = pap.tile([128, 136], DT.float32, tag="pa",
                                           bufs=8, name=f"paf_{jm}")
                        for k in range(8):
                            for gi in range(3):
                                nc.tensor.matmul(
                                    ps[gi][:],
                                    WAg[jm][:, k, 128 * gi:128 * (gi + 1)],
                                    xsq[b4][:, k, :],
                                    start=(k == 0), stop=(k == 7))
                            if b4 == 3:
                                nc.tensor.matmul(
                                    psf[:],
                                    WAg[jm][:, k, 384:512],
                                    xsq[3][:, k, 138:274],
                                    start=(k == 0), stop=(k == 7))
                        # leaf elementwise drains
                        lw = QW if b4 < 3 else 138
                        cb = QW * b4
                        si = drp.tile([128, QW], F16, tag="dr", bufs=8,
                                      name=f"si_{jm}_{b4}")
                        tu = drp.tile([128, QW], F16, tag="dr", bufs=8,
                                      name=f"tu_{jm}_{b4}")
                        nc.scalar.activation(si[:, 0:lw], ps[0][:, 0:lw],
                                             AF.Sigmoid,
                                             bias=BT_t[:, jm:jm + 1])
                        nc.scalar.activation(tu[:, 0:lw], ps[2][:, 0:lw],
                                             AF.Tanh,
                                             bias=BT_t[:, jm + 16:jm + 17])
                        ct = stp.tile([128, QW], F16, tag="st", bufs=4,
                                      name=f"ct_{jm}_{b4}")
                        nc.vector.tensor_mul(ct[:, 0:lw], si[:, 0:lw],
                                             tu[:, 0:lw])

                        def spill_write(state, src):
                            a, b = cb, cb + lw
                            if a < 448:
                                e = min(b, 448)
                                nc.sync.dma_start(
                                    spill4[state, :, jm, a:e],
                                    src[:, 0:e - a])
                            if b > 448:
                                a2 = max(a, 448)
                                nc.sync.dma_start(
                                    spill3[state, :, jm, a2 - 448:b - 448],
                                    src[:, a2 - cb:b - cb])
                        spill_write(0, ct)
                        tanc = drp.tile([128, QW], F16, tag="dr", bufs=8,
                                        name=f"tanc_{jm}_{b4}")
                        nc.scalar.activation(tanc[:, 0:lw], ct[:, 0:lw],
                                             AF.Tanh)
                        so = drp.tile([128, QW], F16, tag="dr", bufs=8,
                                      name=f"so_{jm}_{b4}")
                        nc.scalar.activation(so[:, 0:lw], ps[1][:, 0:lw],
                                             AF.Sigmoid,
                                             bias=BT_t[:, jm + 8:jm + 9])
                        ht = stp.tile([128, QW], F16, tag="st", bufs=4,
                                      name=f"ht_{jm}_{b4}")
                        nc.vector.tensor_mul(ht[:, 0:lw], so[:, 0:lw],
                                             tanc[:, 0:lw])
                        spill_write(1, ht)
                        # parent drains + transpose to node-major (cols
                        # 138..274 of quarter 3 = parent cols 960..1095)
                        if b4 == 3:
                            for gi in range(4):
                                j = jm + 8 * gi
                                bcol = j if gi < 3 else 24 + jm
                                src = (ps[gi][:, 138:274] if gi < 3
                                       else psf[:, 0:136])
                                fm = drp.tile([128, 136], F16, tag="fm",
                                              bufs=4, name=f"fm_{jm}_{gi}")
                                nc.scalar.activation(
                                    fm[:, 0:136], src, AF.Identity,
                                    bias=BT_t[:, bcol:bcol + 1])
                                for half in range(2):
                                    qn = 128 if half == 0 else 8
                                    tp = pap.tile([128, 136], F16,
                                                  tag="pa", bufs=8,
                                                  name=f"tp_{jm}_{gi}_{half}")
                                    nc.tensor.transpose(
                                        tp[0:qn, 0:128],
                                        fm[:, 128 * half:128 * half + qn],
                                        I_t[:, :])
                                    tsb = drp.tile([128, 136], F16,
                                                   tag="tsb", bufs=4,
                                                   name=f"tsb_{jm}_{gi}_{half}")
                                    nc.vector.tensor_copy(tsb[0:qn, 0:128],
                                                          tp[0:qn, 0:128])
                                    nc.sync.dma_start(
                                        nm_dram[128 * half:128 * half + qn,
                                                128 * j:128 * (j + 1)],
                                        tsb[0:qn, 0:128])
                        if b4 == 1 and jm >= 2:
                            load_wr(jm - 2)
                # zero the single real pad-child column (local col 447:
                # node 1023's 8th child on core 7; harmless on other cores)
                load_wr(6)
                load_wr(7)
                nc.gpsimd.dma_start(spill4[0, :, :, 447], zpad[:, :])
                nc.gpsimd.dma_start(spill4[1, :, :, 447], zpad[:, :])

            # ---------------- Rounds ----------------
            import os as _os
            if _os.environ.get('PHASE_A_ONLY'):
                dum = pp.tile([128, 8], DT.float32, name="dum")
                nc.vector.memset(dum[:], 0.0)
                nc.sync.dma_start(
                    rc_d[0, :].rearrange("(m p) -> p m", p=128), dum[:])
                nc.sync.dma_start(
                    rh_d[0, :].rearrange("(m p) -> p m", p=128), dum[:])
                nc.compile()
                return nc
            with (
                tc.tile_pool(name="rps", bufs=1, space="PSUM") as rps,
                tc.tile_pool(name="chp", bufs=1) as chp,
                tc.tile_pool(name="nmp", bufs=1) as nmp,
                tc.tile_pool(name="rwp", bufs=1) as rwp,
                tc.tile_pool(name="sink", bufs=1) as sink,
            ):
                c3_c = sink.tile([128, 8, 64], F16)
                c3_h = sink.tile([128, 8, 64], F16)
                c2_c = sink.tile([128, 8, 8], F16)
                c2_h = sink.tile([128, 8, 8], F16)
                c1_c = sink.tile([128, 8, 8], F16)
                c1_h = sink.tile([128, 8, 8], F16)
                st4_c = sink.tile([128, 8, 56], F16)
                st4_h = sink.tile([128, 8, 56], F16)
                st1_c = sink.tile([128, 8, 1], F16)
                st1_h = sink.tile([128, 8, 1], F16)
                rootc_sb = sink.tile([128, 8], DT.float32)
                rooth_sb = sink.tile([128, 8], DT.float32)

                def group8_sum(prod_ap, out_ap, nb, rn, jm):
                    """out[p, n] = sum_k prod[p, 8n + k]."""
                    a = prod_ap.rearrange("p (n k) -> p n k", k=8)
                    l1 = rwp.tile([128, 256], F16, tag="lvl1", bufs=2,
                                  name=f"l1_{rn}_{jm}")
                    l1v = l1[:, 0:nb * 4].rearrange("p (n k) -> p n k", k=4)
                    nc.vector.tensor_add(l1v, a[:, :, 0:4], a[:, :, 4:8])
                    l2 = rwp.tile([128, 128], F16, tag="lvl2", bufs=2,
                                  name=f"l2_{rn}_{jm}")
                    l2v = l2[:, 0:nb * 2].rearrange("p (n k) -> p n k", k=2)
                    nc.vector.tensor_add(l2v, l1v[:, :, 0:2], l1v[:, :, 2:4])
                    # out[p, n] = l2[p, 2n] + l2[p, 2n+1]  (stride-2 views)
                    e0 = l2v[:, :, 0:1].rearrange("p n k -> p (n k)")
                    e1 = l2v[:, :, 1:2].rearrange("p n k -> p (n k)")
                    nc.vector.tensor_add(out_ap, e0, e1)

                def run_round(rn, get_chC, get_chH, out_c, out_h):
                    nb = NB[rn]
                    q0 = Q0[rn]
                    w8 = 8 * nb
                    # 1. csum (feature-major)
                    csumT = rwp.tile([128, 8, 64], F16, tag="csum",
                                     bufs=1, name=f"csum_{rn}")
                    for m in range(8):
                        group8_sum(get_chC(m), csumT[:, m, 0:nb], nb, rn, m)
                    # 2. iou psum [nb, 3072] node-major
                    ips = rps.tile([nb, 3072], DT.float32, tag="iou", bufs=1,
                                   name=f"ips_{rn}")
                    for k in range(8):
                        for ch in range(6):
                            nc.tensor.matmul(
                                ips[:, 512 * ch:512 * (ch + 1)],
                                csumT[:, k, 0:nb],
                                WRg[ch][:, k, :],
                                start=(k == 0), stop=False)
                    ioux_t = nmp.tile([64, 3072], F16, tag="nm", bufs=1,
                                      name=f"ioux_{rn}")
                    nc.sync.dma_start(ioux_t[0:nb, :], nm_dram[q0:q0 + nb, 0:3072])
                    for ch in range(6):
                        nc.tensor.matmul(
                            ips[:, 512 * ch:512 * (ch + 1)],
                            I_t[0:nb, 0:nb],
                            ioux_t[0:nb, 512 * ch:512 * (ch + 1)],
                            start=False, stop=(ch == 5))
                    # 3-5. f gates (feature-major), prod, fc
                    fxb_t = nmp.tile([64, 1024], F16, tag="nm", bufs=1,
                                     name=f"fxb_{rn}")
                    nc.sync.dma_start(fxb_t[0:nb, :], nm_dram[q0:q0 + nb, 3072:4096])
                    fcT = rwp.tile([128, 8, 64], F16, tag="fcT", bufs=1,
                                   name=f"fcT_{rn}")
                    for j in range(8):
                        fps = rps.tile([128, 512], DT.float32, tag="fp", bufs=2,
                                       name=f"fps_{rn}_{j}")
                        for k in range(8):
                            nc.tensor.matmul(
                                fps[:, 0:w8],
                                WRg[6 + j // 4][:, k,
                                                128 * (j % 4):128 * (j % 4 + 1)],
                                get_chC(k)[:, 0:w8],
                                start=(k == 0), stop=False)
                        nc.tensor.matmul(
                            fps[:, 0:w8],
                            fxb_t[0:nb, 128 * j:128 * (j + 1)],
                            SEL_t[0:nb, 0:w8],
                            start=False, stop=True)
                        fsb = rwp.tile([128, 512], F16, tag="fsb", bufs=2,
                                       name=f"fsb_{rn}_{j}")
                        nc.scalar.activation(fsb[:, 0:w8], fps[:, 0:w8], AF.Sigmoid)
                        prod = rwp.tile([128, 512], F16, tag="fsb", bufs=2,
                                        name=f"prod_{rn}_{j}")
                        nc.vector.tensor_mul(prod[:, 0:w8], fsb[:, 0:w8],
                                             get_chH(j)[:, 0:w8])
                        group8_sum(prod[:, 0:w8], fcT[:, j, 0:nb], nb, rn, 100 + j)
                    # 6. gates from iou psum (node-major)
                    si = rwp.tile([64, 1024], F16, tag="g", bufs=3,
                                  name=f"si_{rn}")
                    tu = rwp.tile([64, 1024], F16, tag="g", bufs=3,
                                  name=f"tu_{rn}")
                    nc.scalar.activation(si[0:nb, :], ips[:, 0:1024], AF.Sigmoid)
                    nc.scalar.activation(tu[0:nb, :], ips[:, 2048:3072], AF.Tanh)
                    p1 = rwp.tile([64, 1024], F16, tag="g", bufs=3,
                                  name=f"p1_{rn}")
                    nc.vector.tensor_mul(p1[0:nb, :], si[0:nb, :], tu[0:nb, :])
                    so = rwp.tile([64, 1024], F16, tag="g", bufs=3,
                                  name=f"so_{rn}")
                    nc.scalar.activation(so[0:nb, :], ips[:, 1024:2048], AF.Sigmoid)
                    # 7-8. transpose to feature-major, combine
                    tw = max(2, nb)
                    for m in range(8):
                        tp1 = rps.tile([128, 64], F16, tag="fp", bufs=2,
                                       name=f"tp1_{rn}_{m}")
                        nc.tensor.transpose(tp1[:, 0:tw],
                                            p1[0:nb, 128 * m:128 * (m + 1)],
                                            I_t[0:nb, 0:tw])
                        cm = out_c(m)
                        nc.vector.tensor_add(cm, tp1[:, 0:nb], fcT[:, m, 0:nb])
                        tso = rps.tile([128, 64], F16, tag="fp", bufs=2,
                                       name=f"tso_{rn}_{m}")
                        nc.tensor.transpose(tso[:, 0:tw],
                                            so[0:nb, 128 * m:128 * (m + 1)],
                                            I_t[0:nb, 0:tw])
                        tanc = rwp.tile([128, 64], F16, tag="tanc", bufs=2,
                                        name=f"tanc_{rn}_{m}")
                        nc.scalar.activation(tanc[:, 0:nb], cm, AF.Tanh)
                        nc.vector.tensor_mul(out_h(m), tso[:, 0:nb],
                                             tanc[:, 0:nb])

                def dram_ch(state, c0, w8, rn):
                    tiles = {}

                    def get(m):
                        if m not in tiles:
                            t = chp.tile([128, 512], F16, tag=f"ch{state}",
                                         bufs=8 if state == 0 else 4,
                                         name=f"ch{state}_{rn}_{m}")
                            nc.sync.dma_start(t[:, 0:w8],
                                              hc_spill[state, m, :, c0:c0 + w8])
                            tiles[m] = t
                        return tiles[m][:, 0:w8]
                    return get

                # ---- R4 ----
                run_round(0,
                          dram_ch(0, 0, 448, 0), dram_ch(1, 0, 448, 0),
                          lambda m: st4_c[:, m, 0:56],
                          lambda m: st4_h[:, m, 0:56])
                nc.sync.dma_start(ag_in[0].rearrange("m p w -> p m w"), st4_c[:])
                nc.sync.dma_start(ag_in[1].rearrange("m p w -> p m w"), st4_h[:])
                nc.gpsimd.collective_compute(
                    "AllGather", mybir.AluOpType.bypass, replica_groups=RG,
                    ins=[ag_in.opt()], outs=[ag_out.opt()])
                pid = nc.gpsimd.partition_id()
                with tc.If(pid == 0):
                    for r in range(NCORES):
                        off = 448 + (S4[r] - 585)
                        nc.gpsimd.dma_start(
                            hc_spill[:, :, :, off:off + C4[r]],
                            ag_out[r, :, :, :, 0:C4[r]])
                # ---- R3 ----
                run_round(1,
                          dram_ch(0, 448, 512, 1), dram_ch(1, 448, 512, 1),
                          lambda m: c3_c[:, m, 0:64],
                          lambda m: c3_h[:, m, 0:64])
                # ---- R2 ----
                run_round(2,
                          lambda m: c3_c[:, m, :], lambda m: c3_h[:, m, :],
                          lambda m: c2_c[:, m, 0:8],
                          lambda m: c2_h[:, m, 0:8])
                # ---- R1 ----
                run_round(3,
                          lambda m: c2_c[:, m, :], lambda m: c2_h[:, m, :],
                          lambda m: st1_c[:, m, 0:1],
                          lambda m: st1_h[:, m, 0:1])
                nc.sync.dma_start(agb_in[0].rearrange("m p w -> p m w"), st1_c[:])
                nc.sync.dma_start(agb_in[1].rearrange("m p w -> p m w"), st1_h[:])
                nc.gpsimd.collective_compute(
                    "AllGather", mybir.AluOpType.bypass, replica_groups=RG,
                    ins=[agb_in.opt()], outs=[agb_out.opt()])
                for r in range(NCORES):
                    nc.sync.dma_start(c1_c[:, :, r:r + 1],
                                      agb_out[r, 0].rearrange("m p w -> p m w"))
                    nc.sync.dma_start(c1_h[:, :, r:r + 1],
                                      agb_out[r, 1].rearrange("m p w -> p m w"))
                # ---- R0 ----
                run_round(4,
                          lambda m: c1_c[:, m, :], lambda m: c1_h[:, m, :],
                          lambda m: rootc_sb[:, m:m + 1],
                          lambda m: rooth_sb[:, m:m + 1])
                nc.sync.dma_start(
                    rc_d[0, :].rearrange("(m p) -> p m", p=128), rootc_sb[:])
                nc.sync.dma_start(
                    rh_d[0, :].rearrange("(m p) -> p m", p=128), rooth_sb[:])

    nc.compile()
    return nc


def _preprocess(inputs, children, w_ioux, b_ioux, w_iouh, b_iouh,
                w_fx, b_fx, w_fh, b_fh):
    f32 = np.float32
    f16 = np.float16
    inputs = np.ascontiguousarray(inputs, dtype=f32)
    b_tot = (np.asarray(b_ioux) + np.asarray(b_iouh)).astype(f32)
    b_fhx = (np.asarray(b_fx) + np.asarray(b_fh)).astype(f32)

    X = inputs.T                                           # [1024, 8192]
    Wcat = np.concatenate([np.asarray(w_ioux, dtype=f32),
                           np.asarray(w_fx, dtype=f32)], axis=0)   # [4096, 1024]
    WcatT = Wcat.T.astype(f16)                             # [1024, 4096]
    # WAg[jm][p, k, gi*128+c] = WcatT[128k+p, 128*(jm+8*gi)+c]
    WAG = np.ascontiguousarray(
        WcatT.reshape(8, 128, 4, 8, 128).transpose(3, 1, 0, 2, 4)
        .reshape(8, 128, 4096))
    WRcat = np.concatenate([np.asarray(w_iouh, dtype=f32).T,
                            np.asarray(w_fh, dtype=f32).T], axis=1)  # [1024, 4096]
    # WRg[ch][p, k, c] = WRcat[128k+p, 512*ch+c]
    WRG = np.ascontiguousarray(
        WRcat.astype(f16).reshape(8, 128, 8, 512).transpose(2, 1, 0, 3)
        .reshape(8, 128, 4096))
    BT = np.empty((128, 32), dtype=f32)
    BT[:, 0:24] = b_tot.reshape(24, 128).T
    BT[:, 24:32] = b_fhx.reshape(8, 128).T
    SEL = np.zeros((128, 1024), dtype=f16)
    q = np.arange(1024)
    SEL[q // 8, q] = 1.0
    I128 = np.eye(128, dtype=f16)

    in_maps = []
    for i in range(NCORES):
        cols = _core_cols(i)
        mask = np.array([c >= 0 for c in cols])
        idx = np.array([max(c, 0) for c in cols])
        Xc = np.where(mask[None, :], X[:, idx], f32(0.0)).astype(f16)  # [1024, 1096]
        # xT[q][p, k*QW + w] = Xc[128k+p, QW*q + w]
        xT = np.ascontiguousarray(
            Xc.reshape(8, 128, 4, QW).transpose(2, 1, 0, 3).reshape(4, 128, 8 * QW))
        in_maps.append(dict(xT=xT, WAG=WAG, WRG=WRG, BT=BT, SEL=SEL, I128=I128))
    return in_maps


def kernel(**inputs):
    nc = _build()
    in_maps = _preprocess(**inputs)
    res = run_bass_kernel_spmd(nc, in_maps, list(range(NCORES))).results
    root_h = np.asarray(res[0]["root_h"], dtype=np.float32)
    root_c = np.asarray(res[0]["root_c"], dtype=np.float32)
    return root_h, root_c
